# revision 1
# baseline (speedup 1.0000x reference)
"""Trainium2 Bass kernel for nn_Attention_48687749267827.

Dense transformer attention block (1x1-conv QKV + windowed relative-position
bias + softmax + 1x1-conv out-proj + layer-scale), data-parallel over batch
across 8 NeuronCores (2 batches per core).

Design notes (per core):
  * Attention is computed in transposed orientation: S^T[m, n] tiles of
    [112, 784] so that the AV product needs no on-chip transposes.  The
    softmax denominator falls out of an extra ones-column appended to V^T.
  * The relative-position bias B[n, m] = table[(rn-rm+27)*55 + (cn-cm+27)]
    is added on the TensorEngine with an identity matmul (start=False PSUM
    accumulation).  Its rhs reads a per-partition *shifted replica* of the
    flattened 55x55 table: partition p (key position m) holds
    db[shift(m) : shift(m)+1513], loaded by strided DMA (the shift is
    affine in (rm, cm), so 4 plain DMA descriptsingle calls per tile do it).
    The [rn, cn] window view of that replica IS the bias tile - no gather.
  * softmax skips the max-subtraction (logits are O(1) here), so
    P = exp(S^T + B^T) directly on ScalarE, written as bf16.
  * All matmuls are bf16 with fp32 PSUM accumulation.
"""

import os
import sys

for _p in ("/opt/trn_rl_repo", "/root/.axon_site/_ro/trn_rl_repo"):
    if os.path.isdir(_p) and _p not in sys.path:
        sys.path.insert(0, _p)

from contextlib import ExitStack

import numpy as np

import concourse.bass as bass
import concourse.tile as tile
import concourse.mybir as mybir
from concourse import bacc
from concourse.bass import ds, ts
from concourse.masks import make_identity

# ---------------------------------------------------------------- constants
B, C_IN, H, W = 16, 384, 28, 28
NUM_HEADS, HEAD_DIM = 12, 32
MID = NUM_HEADS * HEAD_DIM  # 384
OUT = 384
SCALE = HEAD_DIM ** -0.5
N = H * W                   # 784
NCORES = 8
BPC = B // NCORES           # 2 batches per core
DD = 2 * H - 1              # 55
NBIAS = DD * DD             # 3025
MT = 112                    # m-tile rows (4 rm-rows x 28 cm)
NMT = N // MT               # 7
REPW = (H - 1) * DD + (W - 1) + 1   # 1513 window length per partition
NC0, NC1 = 392, 392         # n-chunks (14*28 each, bank-aligned via padded psum)

F32 = mybir.dt.float32
BF16 = mybir.dt.bfloat16

AOP = mybir.AluOpType
AFT = mybir.ActivationFunctionType


def _build_program():
    nc = bacc.Bacc("TRN2", target_bir_lowering=False, debug=False)

    # ------------------------------------------------ DRAM I/O declarations
    x_d = nc.dram_tensor("x", [BPC, C_IN, N], F32, kind="ExternalInput")
    wqT_d = nc.dram_tensor("wqT", [C_IN, MID], F32, kind="ExternalInput")
    wkT_d = nc.dram_tensor("wkT", [C_IN, MID], F32, kind="ExternalInput")
    wvT_d = nc.dram_tensor("wvT", [C_IN, MID], F32, kind="ExternalInput")
    wpT0_d = nc.dram_tensor("wpT0", [768, OUT], F32, kind="ExternalInput")
    wpT1_d = nc.dram_tensor("wpT1", [768, OUT], F32, kind="ExternalInput")
    bq_d = nc.dram_tensor("bq", [MID], F32, kind="ExternalInput")
    bk_d = nc.dram_tensor("bk", [MID], F32, kind="ExternalInput")
    bp_d = nc.dram_tensor("bp", [OUT], F32, kind="ExternalInput")
    gm_d = nc.dram_tensor("gm", [OUT], F32, kind="ExternalInput")
    db_d = nc.dram_tensor("db", [NUM_HEADS, NBIAS], F32, kind="ExternalInput")
    out_d = nc.dram_tensor("out", [BPC, OUT, N], F32, kind="ExternalOutput")

    with ExitStack() as ctx:
        tc = ctx.enter_context(tile.TileContext(nc))
        const = ctx.enter_context(tc.tile_pool(name="const", bufs=1))
        dram = ctx.enter_context(tc.tile_pool(name="dram", bufs=1, space="DRAM"))
        stage = ctx.enter_context(tc.tile_pool(name="stage", bufs=2))

        # ---------------------------------------- phase 0: weights & tables
        def load_cast(dsrc, shape3, tag):
            w = stage.tile(shape3, F32, tag="wstage")
            nc.sync.dma_start(w[:], dsrc[:].rearrange("(a p) m -> p a m", p=128))
            o = const.tile(shape3, BF16, tag=tag)
            nc.vector.tensor_copy(o[:], w[:])
            return o

        wqT = load_cast(wqT_d, [128, 3, MID], "wqT")
        wkT = load_cast(wkT_d, [128, 3, MID], "wkT")
        wvT = load_cast(wvT_d, [128, 3, MID], "wvT")
        wpT = [load_cast(wpT0_d, [128, 6, OUT], "wpT0"),
               load_cast(wpT1_d, [128, 6, OUT], "wpT1")]

        def load_vec(dsrc, cols, tag):
            o = const.tile([128, cols], F32, tag=tag)
            nc.sync.dma_start(o[:], dsrc[:].rearrange("(a p) -> p a", p=128))
            return o

        bq_sb = load_vec(bq_d, 3, "bq")
        bk_sb = load_vec(bk_d, 3, "bk")
        bp_sb = load_vec(bp_d, 3, "bp")
        gm_sb = load_vec(gm_d, 3, "gm")

        # bias table -> bf16 replica source in DRAM
        dbf = stage.tile([NUM_HEADS, NBIAS], F32, tag="dbstage")
        nc.sync.dma_start(dbf[:], db_d[:])
        dbb = const.tile([NUM_HEADS, NBIAS], BF16, tag="dbb")
        nc.vector.tensor_copy(dbb[:], dbf[:])
        db_bf = dram.tile([NUM_HEADS, NBIAS], BF16, tag="db_bf")
        nc.sync.dma_start(db_bf[:], dbb[:])
        # band table: db_ext[h, cm, t] = db[h, t - cm]  (all-positive-stride
        # source for the per-partition shifted replica loads below)
        EXTW = NBIAS + W                     # 3053
        db_ext = dram.tile([NUM_HEADS, W, EXTW], BF16, tag="db_ext")
        for cm in range(W):
            nc.sync.dma_start(db_ext[:, cm, cm:cm + NBIAS], db_bf[:])

        ident = const.tile([MT, MT], BF16, tag="ident")
        make_identity(nc, ident[:])

        # HAM warm-up: ~6us of dense dummy matmuls at program start flips the
        # PE clock gate to 8/8 while the input DMAs are still in flight.
        warm = const.tile([128, 512], BF16, tag="warm")
        warmout = const.tile([128, 512], F32, tag="warmout")
        nc.vector.memset(warm[:], 0.0)

        # ---------------------------------------- per-batch persistent sbuf
        xf = [const.tile([128, 3, N], BF16, tag=f"xf{b}", name=f"xf{b}") for b in range(BPC)]
        q_sb = [const.tile([128, 3, N], BF16, tag=f"q{b}", name=f"q{b}") for b in range(BPC)]
        k_sb = [const.tile([128, 3, N], BF16, tag=f"k{b}", name=f"k{b}") for b in range(BPC)]
        vT = [const.tile([MT, NMT, NUM_HEADS, 2 * HEAD_DIM], BF16, tag=f"vT{b}",
                          name=f"vT{b}") for b in range(BPC)]
        omid = [const.tile([128, 6, N], BF16, tag=f"om{b}", name=f"om{b}") for b in range(BPC)]

        for b in range(BPC):
            xs = stage.tile([128, 3, N], F32, tag="xstage")
            nc.sync.dma_start(xs[:], x_d[b].rearrange("(a p) n -> p a n", p=128))
            nc.vector.tensor_copy(xf[b][:], xs[:])
            nc.gpsimd.memset(omid[b][:], 0.0)
            if b == 0:
                nc.vector.memset(vT[b][:, :, :, HEAD_DIM:], 1.0)
            else:
                nc.vector.memset(vT[b][:, :, :, :HEAD_DIM], 1.0)

        NCHUNKS = ((0, 512), (512, N - 512))

        # ------------------------------------------- phase 1: q, k, v^T
        with tc.tile_pool(name="pp1", bufs=2, space="PSUM") as pp1:
            wps = pp1.tile([128, 512], F32, tag="warmps", name="warmps")
            for wi in range(16):
                nc.tensor.matmul(wps[:], lhsT=warm[:, :128], rhs=warm[:],
                                 start=True, stop=True)
            nc.vector.tensor_copy(warmout[:], wps[:])
            for b in range(BPC):
                for mo in range(3):
                    ps = pp1.tile([128, 2, 512], F32, tag="ps")
                    for kc in range(3):
                        for c, (n0, nn) in enumerate(NCHUNKS):
                            nc.tensor.matmul(
                                ps[:, c, :nn],
                                lhsT=wqT[:, kc, ts(mo, 128)],
                                rhs=xf[b][:, kc, n0:n0 + nn],
                                start=(kc == 0), stop=(kc == 2))
                    for c, (n0, nn) in enumerate(NCHUNKS):
                        nc.vector.tensor_scalar(
                            q_sb[b][:, mo, n0:n0 + nn], ps[:, c, :nn],
                            bq_sb[:, mo:mo + 1], SCALE, AOP.add, AOP.mult)
                for mo in range(3):
                    ps = pp1.tile([128, 2, 512], F32, tag="ps")
                    for kc in range(3):
                        for c, (n0, nn) in enumerate(NCHUNKS):
                            nc.tensor.matmul(
                                ps[:, c, :nn],
                                lhsT=wkT[:, kc, ts(mo, 128)],
                                rhs=xf[b][:, kc, n0:n0 + nn],
                                start=(kc == 0), stop=(kc == 2))
                    for c, (n0, nn) in enumerate(NCHUNKS):
                        nc.vector.tensor_scalar(
                            k_sb[b][:, mo, n0:n0 + nn], ps[:, c, :nn],
                            bk_sb[:, mo:mo + 1], None, AOP.add)
                for nt in range(NMT):
                    ps2 = pp1.tile([MT, MID], F32, tag="ps2")
                    for kc in range(3):
                        nc.tensor.matmul(
                            ps2[:],
                            lhsT=xf[b][:, kc, ts(nt, MT)],
                            rhs=wvT[:, kc, :],
                            start=(kc == 0), stop=(kc == 2))
                    vdst = (vT[b][:, nt, :, :HEAD_DIM] if b == 0
                            else vT[b][:, nt, :, HEAD_DIM:])
                    nc.vector.tensor_copy(
                        vdst,
                        ps2[:].rearrange("p (h d) -> p h d", h=NUM_HEADS))

        # ------------------------------------------- phase 2: attention
        with tc.tile_pool(name="spool", bufs=2, space="PSUM") as spool, \
             tc.tile_pool(name="avpool", bufs=2, space="PSUM") as avpool, \
             tc.tile_pool(name="rep", bufs=6) as reppool, \
             tc.tile_pool(name="pt", bufs=6) as ptpool, \
             tc.tile_pool(name="drp", bufs=2) as drpool:
            db_ap = db_ext[:]
            EXTW = NBIAS + W
            for t in range(6):                      # head pairs (2t, 2t+1)
                avt = [avpool.tile([128, 2, 512], F32, tag="av", name=f"av{t}_{bb}")
                       for bb in range(BPC)]
                for mt in range(NMT):
                    reps = []
                    for j in range(2):
                        h = 2 * t + j
                        rp = reppool.tile([MT, H, DD], BF16, tag="rep")
                        rpf = rp[:].rearrange("p a b -> p (a b)")
                        for a in range(4):
                            rm = 4 * mt + a
                            off = (h * W * EXTW
                                   + (H - 1 - rm) * DD + (W - 1))
                            src = bass.AP(
                                tensor=db_ap.tensor,
                                offset=db_ap.offset + off,
                                ap=[[EXTW, W], [1, REPW]])
                            nc.sync.dma_start(rpf[28 * a:28 * a + 28, :REPW], src)
                        reps.append(rp)
                    for b in range(BPC):
                        pts = []
                        for j in range(2):
                            h = 2 * t + j
                            hb, hc = 32 * (h % 4), h // 4
                            s_t = spool.tile([MT, 2, 512], F32, tag="s")
                            for c in range(2):
                                n0 = c * NC0
                                nc.tensor.matmul(
                                    s_t[:, c, :NC0],
                                    lhsT=k_sb[b][ds(hb, 32), hc, ts(mt, MT)],
                                    rhs=q_sb[b][ds(hb, 32), hc, n0:n0 + NC0],
                                    start=True, stop=False,
                                    tile_position=(hb, 0))
                                nc.tensor.matmul(
                                    s_t[:, c, :NC0],
                                    lhsT=ident[:],
                                    rhs=reps[j][:, 14 * c:14 * c + 14, :W],
                                    start=False, stop=True)
                            pt = ptpool.tile([MT, N], BF16, tag="pt")
                            nc.scalar.activation(
                                pt[:].rearrange("p (c n) -> p c n", c=2),
                                s_t[:, :, :NC0], AFT.Exp)
                            pts.append(pt)
                        for j in range(2):
                            h = 2 * t + j
                            for c in range(2):
                                n0 = c * NC0
                                nc.tensor.matmul(
                                    avt[b][ds(64 * j, 64), c, :NC0],
                                    lhsT=vT[b][:, mt, h, :],
                                    rhs=pts[j][:, n0:n0 + NC0],
                                    start=(mt == 0), stop=(mt == NMT - 1),
                                    skip_group_check=True)
                # normalize: omid rows = av[0:32] * (1/D), D = av row 32
                drec = drpool.tile([128, N], F32, tag="drec")
                drecR = drpool.tile([128, N], F32, tag="drecR")
                avsb = [drpool.tile([128, N], F32, tag=f"avs{bb}",
                                    name=f"avs{t}_{bb}") for bb in range(BPC)]
                for b in range(BPC):
                    nc.vector.tensor_copy(
                        avsb[b][:].rearrange("p (c n) -> p c n", c=2),
                        avt[b][:, :, :NC0])
                    for j in range(2):
                        srcrow = 64 * j + (32 if b == 0 else 0)
                        dstrow = 64 * j + 32 * b
                        nc.sync.dma_start(
                            drec[ds(dstrow, 32), :],
                            avsb[b][ds(srcrow, 32), :])
                nc.vector.reciprocal_approx_fast(drecR[:], drec[:])
                for b in range(BPC):
                    for j in range(2):
                        orow = 64 * j + 32 * b
                        nc.vector.tensor_tensor(
                            omid[b][ds(orow, 32), t, :],
                            avsb[b][ds(orow, 32), :],
                            drecR[ds(orow, 32), :],
                            AOP.mult)

        # ------------------------------------------- phase 3: out-projection
        with tc.tile_pool(name="pp3", bufs=2, space="PSUM") as pp3, \
             tc.tile_pool(name="osb", bufs=2) as osb:
            for b in range(BPC):
                for oc in range(3):
                    ps = pp3.tile([128, 2, 512], F32, tag="po")
                    for kc in range(6):
                        for c, (n0, nn) in enumerate(NCHUNKS):
                            nc.tensor.matmul(
                                ps[:, c, :nn],
                                lhsT=wpT[b][:, kc, ts(oc, 128)],
                                rhs=omid[b][:, kc, n0:n0 + nn],
                                start=(kc == 0), stop=(kc == 5))
                    o_t = osb.tile([128, N], F32, tag="ot")
                    for c, (n0, nn) in enumerate(NCHUNKS):
                        nc.vector.tensor_scalar(
                            o_t[:, n0:n0 + nn], ps[:, c, :nn],
                            bp_sb[:, oc:oc + 1], gm_sb[:, oc:oc + 1],
                            AOP.add, AOP.mult)
                    nc.sync.dma_start(out_d[b, ts(oc, 128), :], o_t[:])

    nc.compile()
    return nc


_NC_CACHE = None


def _get_program():
    global _NC_CACHE
    if _NC_CACHE is None:
        _NC_CACHE = _build_program()
    return _NC_CACHE


def _host_prep(inputs):
    """Shard/layout prep (pure slicing / transposition, no math)."""
    x = np.asarray(inputs["x"], np.float32).reshape(B, C_IN, N)
    Wq = np.asarray(inputs["Wq"], np.float32)
    Wkv = np.asarray(inputs["Wkv"], np.float32)
    Wproj = np.asarray(inputs["Wproj"], np.float32)
    bq = np.asarray(inputs["bq"], np.float32)
    bkv = np.asarray(inputs["bkv"], np.float32)
    bproj = np.asarray(inputs["bproj"], np.float32)
    gamma = np.asarray(inputs["gamma"], np.float32)
    bt = np.asarray(inputs["bias_table"], np.float32)

    wqT = np.ascontiguousarray(Wq.T)
    wkT = np.ascontiguousarray(Wkv[:MID].T)
    wvT = np.ascontiguousarray(Wkv[MID:].T)
    WT = np.ascontiguousarray(Wproj.T)          # [mid, out]
    wpT0 = np.zeros((768, OUT), np.float32)     # b0: rows 0-31 / 64-95 per tile
    wpT1 = np.zeros((768, OUT), np.float32)     # b1: rows 32-63 / 96-127
    for t in range(6):
        wpT0[128 * t:128 * t + 32] = WT[64 * t:64 * t + 32]
        wpT0[128 * t + 64:128 * t + 96] = WT[64 * t + 32:64 * t + 64]
        wpT1[128 * t + 32:128 * t + 64] = WT[64 * t:64 * t + 32]
        wpT1[128 * t + 96:128 * t + 128] = WT[64 * t + 32:64 * t + 64]
    db = np.ascontiguousarray(bt.T)             # [heads, 3025]

    shared = {
        "wqT": wqT, "wkT": wkT, "wvT": wvT, "wpT0": wpT0, "wpT1": wpT1,
        "bq": bq, "bk": bkv[:MID],
        "bp": bproj + Wproj @ bkv[MID:], "gm": gamma, "db": db,
    }
    in_maps = []
    for c in range(NCORES):
        m = dict(shared)
        m["x"] = np.ascontiguousarray(x[BPC * c:BPC * (c + 1)])
        in_maps.append(m)
    return in_maps


def kernel(**inputs) -> np.ndarray:
    from concourse.bass_utils import run_bass_kernel_spmd

    nc = _get_program()
    in_maps = _host_prep(inputs)
    res = run_bass_kernel_spmd(nc, in_maps, core_ids=list(range(NCORES)))
    outs = [res.results[c]["out"] for c in range(NCORES)]
    full = np.concatenate(outs, axis=0)          # [16, 384, 784]
    return np.ascontiguousarray(full.reshape(B, OUT, H, W).astype(np.float32))


if __name__ == "__main__":
    prog = _get_program()
    print("program built ok:",
          0, "instructions")



# revision 3
# speedup vs baseline: 1.2802x; 1.2802x over previous
"""Trainium2 Bass kernel v2 for nn_Attention_48687749267827.

Restructured from the 651us baseline around the measured bottleneck
(TensorE 77% busy, cold/unpipelined matmuls, 163us of bias identity
matmuls):

  * QK^T runs 4 heads concurrently via PE row-tiling (tile_position=(32h,0)),
    one 392-col chunk per head per psum bank, no bias matmul in between.
  * The relative-position bias is applied two ways, split by m-tile to
    balance engines: PE route adds raw B via an identity matmul into the
    S psum (start=False); DVE route multiplies P0=exp(S) by E=exp(B) with
    one bf16 2x tensor_tensor per (g,b,mt).
  * Bias replicas stream from device-built compacted DRAM tables
    (tbl[h, cm, dr, cn] -> per-partition contiguous 784-elem runs).
  * exp is batched: one activation over 4 psum banks (FD=1568) per chunk.
  * AV packs 2 heads per bank via col-tiling (tile_position=(0,64j));
    lhsT is [v(32)|ones(1)|zeros(31)] so all 128 psum rows get written
    (keeps the normalize tiles junk-free); denominator comes free.
  * Normalize: fp32 copy of AV psum, D rows broadcast with
    gpsimd.partition_broadcast, reciprocal_approx_fast, one mult.
  * Out-proj uses a host-permuted wpT whose zero rows kill the junk rows
    of omid; epilogue fuses gamma*x+bp on DVE; output bf16 (host casts).
"""

import os
import sys

for _p in ("/opt/trn_rl_repo", "/root/.axon_site/_ro/trn_rl_repo"):
    if os.path.isdir(_p) and _p not in sys.path:
        sys.path.insert(0, _p)

from contextlib import ExitStack

import numpy as np

import concourse.bass as bass
import concourse.tile as tile
import concourse.mybir as mybir
from concourse import bacc
from concourse.bass import ds, ts
from concourse.masks import make_identity

# ---------------------------------------------------------------- constants
B, C_IN, H, W = 16, 384, 28, 28
NUM_HEADS, HEAD_DIM = 12, 32
MID = NUM_HEADS * HEAD_DIM  # 384
OUT = 384
SCALE = HEAD_DIM ** -0.5
N = H * W                   # 784
NCORES = 8
BPC = B // NCORES           # 2 batches per core
DD = 2 * H - 1              # 55
NBIAS = DD * DD             # 3025
MT = 112                    # m-tile rows (4 rm-rows x 28 cm)
NMT = N // MT               # 7
NC = 392                    # n-chunk (bank capacity 512 fp32)
ECW = DD * W                # 1540: per-(h,cm) compacted table width
EHW = W * ECW               # 43120: per-h stride in compacted table
# m-tiles < PE_MT use the PE identity-add bias route; the rest DVE mult.
PE_MT = 3

F32 = mybir.dt.float32
BF16 = mybir.dt.bfloat16

AOP = mybir.AluOpType
AFT = mybir.ActivationFunctionType


def _build_program():
    nc = bacc.Bacc("TRN2", target_bir_lowering=False, debug=False)

    # ------------------------------------------------ DRAM I/O declarations
    x_d = nc.dram_tensor("x", [BPC, C_IN, N], F32, kind="ExternalInput")
    wqT_d = nc.dram_tensor("wqT", [C_IN, MID], F32, kind="ExternalInput")
    wkT_d = nc.dram_tensor("wkT", [C_IN, MID], F32, kind="ExternalInput")
    wvT_d = nc.dram_tensor("wvT", [C_IN, MID], F32, kind="ExternalInput")
    wpT_d = nc.dram_tensor("wpT", [768, OUT], F32, kind="ExternalInput")
    bq_d = nc.dram_tensor("bq", [MID], F32, kind="ExternalInput")
    bk_d = nc.dram_tensor("bk", [MID], F32, kind="ExternalInput")
    bp_d = nc.dram_tensor("bp", [OUT], F32, kind="ExternalInput")
    gm_d = nc.dram_tensor("gm", [OUT], F32, kind="ExternalInput")
    db_d = nc.dram_tensor("db", [NUM_HEADS, NBIAS], F32, kind="ExternalInput")
    out_d = nc.dram_tensor("out", [BPC, OUT, N], BF16, kind="ExternalOutput")

    with ExitStack() as ctx:
        tc = ctx.enter_context(tile.TileContext(nc))
        const = ctx.enter_context(tc.tile_pool(name="const", bufs=1))
        dram = ctx.enter_context(tc.tile_pool(name="dram", bufs=1, space="DRAM"))
        stage = ctx.enter_context(tc.tile_pool(name="stage", bufs=2))

        # ---------------------------------------- phase 0: weights & tables
        def load_cast(dsrc, shape3, tag):
            w = stage.tile(shape3, F32, tag="wstage")
            nc.sync.dma_start(w[:], dsrc[:].rearrange("(a p) m -> p a m", p=128))
            o = const.tile(shape3, BF16, tag=tag)
            nc.vector.tensor_copy(o[:], w[:])
            return o

        wqT = load_cast(wqT_d, [128, 3, MID], "wqT")
        wkT = load_cast(wkT_d, [128, 3, MID], "wkT")
        wvT = load_cast(wvT_d, [128, 3, MID], "wvT")
        wpT = load_cast(wpT_d, [128, 6, OUT], "wpT")

        def load_vec(dsrc, cols, tag):
            o = const.tile([128, cols], F32, tag=tag)
            nc.sync.dma_start(o[:], dsrc[:].rearrange("(a p) -> p a", p=128))
            return o

        bq_sb = load_vec(bq_d, 3, "bq")
        bk_sb = load_vec(bk_d, 3, "bk")
        bp_sb = load_vec(bp_d, 3, "bp")
        gm_sb = load_vec(gm_d, 3, "gm")

        # bias table: raw bf16 (PE add route) + exp'd bf16 (DVE mult route)
        dbb = const.tile([NUM_HEADS, DD, DD], BF16, tag="dbb")
        dbe = const.tile([NUM_HEADS, DD, DD], BF16, tag="dbe")
        db_r = db_d[:].rearrange("h (a b) -> h a b", a=DD)
        for r0, rn in ((0, 28), (28, DD - 28)):
            dbf = stage.tile([NUM_HEADS, 28, DD], F32, tag="dbstage")
            nc.sync.dma_start(dbf[:, :rn, :], db_r[:, r0:r0 + rn, :])
            nc.vector.tensor_copy(dbb[:, r0:r0 + rn, :], dbf[:, :rn, :])
            nc.scalar.activation(dbe[:, r0:r0 + rn, :], dbf[:, :rn, :], AFT.Exp)

        # compacted window tables in DRAM:
        #   tbl[h, cm, dr, cn] = db[h, dr, (27 - cm) + cn]
        # so a rep window (28 dr-rows x 28 cn) per partition is one
        # contiguous 784-elem run starting at row dr0 = 27 - rm.
        db_raw = dram.tile([NUM_HEADS, W, DD, W], BF16, tag="db_raw")
        db_exp = dram.tile([NUM_HEADS, W, DD, W], BF16, tag="db_exp")
        for cm in range(W):
            c0 = W - 1 - cm
            nc.sync.dma_start(db_raw[:, cm], dbb[:, :, c0:c0 + W])
            nc.sync.dma_start(db_exp[:, cm], dbe[:, :, c0:c0 + W])

        ident = const.tile([MT, MT], BF16, tag="ident")
        make_identity(nc, ident[:])
        onebc = const.tile([128, 64], F32, tag="onebc")
        nc.vector.memset(onebc[:], 1.0)

        # HAM warm-up: dense dummy matmuls at program start flip the PE
        # clock gate to 8/8 while the input DMAs are in flight.
        warm = const.tile([128, 512], BF16, tag="warm")
        warmout = const.tile([128, 512], BF16, tag="warmout")
        nc.vector.memset(warm[:], 0.0)

        # ---------------------------------------- per-batch persistent sbuf
        q_sb = [const.tile([128, 3, N], BF16, tag=f"q{b}", name=f"q{b}") for b in range(BPC)]
        k_sb = [const.tile([128, 3, N], BF16, tag=f"k{b}", name=f"k{b}") for b in range(BPC)]
        # AV stationary: [v(32) | ones(1) | zeros(31)] per head -> 64 cols
        vls = [const.tile([MT, NMT, NUM_HEADS, 64], BF16, tag=f"v{b}", name=f"v{b}")
               for b in range(BPC)]
        # omid block kc=2g+hp: rows 0:32 head 2kc vals, 64:96 head 2kc+1
        omid = [const.tile([128, 6, 2, NC], BF16, tag=f"om{b}", name=f"om{b}")
                for b in range(BPC)]

        NCHUNKS = ((0, 512), (512, N - 512))

        # ------------------------------------------- phase 1: q, k, v
        with tc.tile_pool(name="xfp", bufs=1) as xfp, \
             tc.tile_pool(name="pp1", bufs=2, space="PSUM") as pp1, \
             tc.tile_pool(name="pp1v", bufs=2, space="PSUM") as pp1v:
            xf = [xfp.tile([128, 3, N], BF16, tag=f"xf{b}", name=f"xf{b}")
                  for b in range(BPC)]
            for b in range(BPC):
                xs = stage.tile([128, 3, N], F32, tag="xstage")
                nc.sync.dma_start(xs[:], x_d[b].rearrange("(a p) n -> p a n", p=128))
                nc.vector.tensor_copy(xf[b][:], xs[:])
                nc.vector.memset(vls[b][:, :, :, 32:], 0.0)
                nc.vector.memset(vls[b][:, :, :, 32:33], 1.0)
            wps = pp1.tile([128, 512], F32, tag="warmps")
            for wi in range(16):
                nc.tensor.matmul(wps[:], lhsT=warm[:, :128], rhs=warm[:],
                                 start=True, stop=True)
            nc.vector.tensor_copy(warmout[:], wps[:])
            for b in range(BPC):
                for mo in range(3):
                    ps = pp1.tile([128, 2, 512], F32, tag="ps")
                    for kc in range(3):
                        for c, (n0, nn) in enumerate(NCHUNKS):
                            nc.tensor.matmul(
                                ps[:, c, :nn],
                                lhsT=wqT[:, kc, ts(mo, 128)],
                                rhs=xf[b][:, kc, n0:n0 + nn],
                                start=(kc == 0), stop=(kc == 2))
                    for c, (n0, nn) in enumerate(NCHUNKS):
                        nc.vector.tensor_scalar(
                            q_sb[b][:, mo, n0:n0 + nn], ps[:, c, :nn],
                            bq_sb[:, mo:mo + 1], SCALE, AOP.add, AOP.mult)
                for mo in range(3):
                    ps = pp1.tile([128, 2, 512], F32, tag="ps")
                    for kc in range(3):
                        for c, (n0, nn) in enumerate(NCHUNKS):
                            nc.tensor.matmul(
                                ps[:, c, :nn],
                                lhsT=wkT[:, kc, ts(mo, 128)],
                                rhs=xf[b][:, kc, n0:n0 + nn],
                                start=(kc == 0), stop=(kc == 2))
                    for c, (n0, nn) in enumerate(NCHUNKS):
                        nc.vector.tensor_scalar(
                            k_sb[b][:, mo, n0:n0 + nn], ps[:, c, :nn],
                            bk_sb[:, mo:mo + 1], None, AOP.add)
                for nt in range(NMT):
                    ps2 = pp1v.tile([MT, MID], F32, tag="ps2")
                    for kc in range(3):
                        nc.tensor.matmul(
                            ps2[:],
                            lhsT=xf[b][:, kc, ts(nt, MT)],
                            rhs=wvT[:, kc, :],
                            start=(kc == 0), stop=(kc == 2))
                    nc.vector.tensor_copy(
                        vls[b][:, nt, :, :HEAD_DIM],
                        ps2[:].rearrange("p (h d) -> p h d", h=NUM_HEADS))

        # ------------------------------------------- phase 2: attention
        with tc.tile_pool(name="spool", bufs=1, space="PSUM") as spool, \
             tc.tile_pool(name="avpool", bufs=1, space="PSUM") as avpool, \
             tc.tile_pool(name="rep", bufs=7) as reppool, \
             tc.tile_pool(name="pt", bufs=2) as ptpool, \
             tc.tile_pool(name="nrm", bufs=1) as nrmpool:
            for g in range(3):
                # prefetch all 7 rep tiles for this head-group (shared b0/b1)
                reps = []
                for mt in range(NMT):
                    rp = reppool.tile([MT, 4, N], BF16, tag="rep")
                    tbl = db_raw if mt < PE_MT else db_exp
                    for a in range(4):
                        rm = 4 * mt + a
                        src = bass.AP(
                            tensor=tbl[:].tensor,
                            offset=tbl[:].offset + 4 * g * EHW
                                   + (H - 1 - rm) * W,
                            ap=[[ECW, W], [EHW, 4], [1, W * W]])
                        nc.sync.dma_start(rp[ds(28 * a, 28), :, :], src)
                    reps.append(rp)
                for b in range(BPC):
                    avt = avpool.tile([128, 2, 2, 512], F32, tag="av",
                                      name=f"av{g}_{b}")
                    for mt in range(NMT):
                        pe_route = mt < PE_MT
                        s_t = spool.tile([128, 4, 512], F32, tag="s")
                        pts = ptpool.tile([MT, 4, N], BF16, tag="pt")
                        for c in range(2):
                            n0 = c * NC
                            for hh in range(4):
                                nc.tensor.matmul(
                                    s_t[:MT, hh, :NC],
                                    lhsT=k_sb[b][ds(32 * hh, 32), g,
                                                 ts(mt, MT)],
                                    rhs=q_sb[b][ds(32 * hh, 32), g,
                                                n0:n0 + NC],
                                    start=True, stop=not pe_route,
                                    tile_position=(32 * hh, 0))
                            if pe_route:
                                for hh in range(4):
                                    nc.tensor.matmul(
                                        s_t[:MT, hh, :NC],
                                        lhsT=ident[:],
                                        rhs=reps[mt][:, hh, n0:n0 + NC],
                                        start=False, stop=True)
                            nc.scalar.activation(
                                pts[:, :, n0:n0 + NC],
                                s_t[:MT, :, :NC], AFT.Exp)
                        if not pe_route:
                            nc.vector.tensor_tensor(
                                pts[:], pts[:], reps[mt][:], AOP.mult)
                        for c in range(2):
                            n0 = c * NC
                            for hp in range(2):
                                for j in range(2):
                                    h = 4 * g + 2 * hp + j
                                    nc.tensor.matmul(
                                        avt[ds(64 * j, 64), hp, c, :NC],
                                        lhsT=vls[b][:, mt, h, :],
                                        rhs=pts[:, 2 * hp + j, n0:n0 + NC],
                                        start=(mt == 0), stop=(mt == NMT - 1),
                                        tile_position=(0, 64 * j),
                                        skip_group_check=True)
                    # ---- normalize: omid rows = v-rows * (1/D)
                    avnf = nrmpool.tile([128, 2, 2, NC], F32, tag="avnf")
                    drecf = nrmpool.tile([128, 2, 2, NC], F32, tag="drecf")
                    nc.vector.tensor_copy(avnf[:], avt[:, :, :, :NC])
                    # replicate D rows across their 64-row bands with a
                    # ones-column matmul into the (now dead) avt banks
                    for hp in range(2):
                        for c in range(2):
                            for j in range(2):
                                nc.tensor.matmul(
                                    avt[ds(64 * j, 64), hp, c, :NC],
                                    lhsT=onebc[ds(64 * j + 32, 1), :],
                                    rhs=avnf[ds(64 * j + 32, 1), hp, c, :],
                                    start=True, stop=True,
                                    tile_position=(64 * j + 32, 64 * j),
                                    skip_group_check=True)
                    for hp in range(2):
                        nc.vector.reciprocal_approx_fast(
                            drecf[:, hp, :, :], avt[:, hp, :, :NC])
                    nc.vector.tensor_tensor(
                        omid[b][:, ds(2 * g, 2), :, :],
                        avnf[:], drecf[:], AOP.mult)

        # ------------------------------------------- phase 3: out-projection
        with tc.tile_pool(name="pp3", bufs=2, space="PSUM") as pp3, \
             tc.tile_pool(name="osb", bufs=2) as osb:
            for b in range(BPC):
                for oc in range(3):
                    ps = pp3.tile([128, 2, 512], F32, tag="po")
                    for kc in range(6):
                        for c in range(2):
                            nc.tensor.matmul(
                                ps[:, c, :NC],
                                lhsT=wpT[:, kc, ts(oc, 128)],
                                rhs=omid[b][:, kc, c, :],
                                start=(kc == 0), stop=(kc == 5))
                    o_t = osb.tile([128, N], BF16, tag="ot")
                    for c in range(2):
                        nc.vector.tensor_scalar(
                            o_t[:, c * NC:(c + 1) * NC], ps[:, c, :NC],
                            gm_sb[:, oc:oc + 1], bp_sb[:, oc:oc + 1],
                            AOP.mult, AOP.add)
                    nc.sync.dma_start(out_d[b, ts(oc, 128), :], o_t[:])

    nc.compile()
    return nc


_NC_CACHE = None


def _get_program():
    global _NC_CACHE
    if _NC_CACHE is None:
        _NC_CACHE = _build_program()
    return _NC_CACHE


def _host_prep(inputs):
    """Shard/layout prep (pure slicing / transposition, no math)."""
    x = np.asarray(inputs["x"], np.float32).reshape(B, C_IN, N)
    Wq = np.asarray(inputs["Wq"], np.float32)
    Wkv = np.asarray(inputs["Wkv"], np.float32)
    Wproj = np.asarray(inputs["Wproj"], np.float32)
    bq = np.asarray(inputs["bq"], np.float32)
    bkv = np.asarray(inputs["bkv"], np.float32)
    bproj = np.asarray(inputs["bproj"], np.float32)
    gamma = np.asarray(inputs["gamma"], np.float32)
    bt = np.asarray(inputs["bias_table"], np.float32)

    wqT = np.ascontiguousarray(Wq.T)
    wkT = np.ascontiguousarray(Wkv[:MID].T)
    wvT = np.ascontiguousarray(Wkv[MID:].T)
    WT = np.ascontiguousarray(Wproj.T)          # [mid, out]
    wpT = np.zeros((768, OUT), np.float32)
    for kc in range(6):
        wpT[128 * kc:128 * kc + 32] = WT[64 * kc:64 * kc + 32]
        wpT[128 * kc + 64:128 * kc + 96] = WT[64 * kc + 32:64 * kc + 64]
    db = np.ascontiguousarray(bt.T)             # [heads, 3025]

    shared = {
        "wqT": wqT, "wkT": wkT, "wvT": wvT, "wpT": wpT,
        "bq": bq, "bk": bkv[:MID],
        "bp": bproj + Wproj @ bkv[MID:], "gm": gamma, "db": db,
    }
    in_maps = []
    for c in range(NCORES):
        m = dict(shared)
        m["x"] = np.ascontiguousarray(x[BPC * c:BPC * (c + 1)])
        in_maps.append(m)
    return in_maps


def kernel(**inputs) -> np.ndarray:
    from concourse.bass_utils import run_bass_kernel_spmd

    nc = _get_program()
    in_maps = _host_prep(inputs)
    res = run_bass_kernel_spmd(nc, in_maps, core_ids=list(range(NCORES)))
    outs = [np.asarray(res.results[c]["out"], np.float32)
            for c in range(NCORES)]
    full = np.concatenate(outs, axis=0)          # [16, 384, 784]
    return np.ascontiguousarray(full.reshape(B, OUT, H, W))


if __name__ == "__main__":
    prog = _get_program()
    print("program built ok")
